# revision 15
# baseline (speedup 1.0000x reference)
"""Trainium2 Bass kernel for nn_CPA_CCA_block (channel attention + spatial attention + fusion).

Batch-sharded: 8 samples over 8 NeuronCores, replicated weights, zero collectives.
Key tricks:
  - out1/out2 never materialized: w_h @ [out1;out2] folded on host into
    (w_h1@w_beta)@E + (w_h2@w_e)@Esp + (w_h1+w_h2)@x
  - channel-attention softmax logits computed in full fp32 (logits ~N(0,464))
    via PE-transposed x chunks; fat matmuls run in float32r (fp32 with 11-bit
    mantissa, full PE rate); tiny spatial branch in bf16.
  - fusion stage streamed in 480-pixel chunks; Esp kept in (w,h) free order, read
    back through a permuted AP so no output-side transpose is needed.
  - lrelu = (x*0.001) max x on DVE (HW Lrelu activation ignores alpha).
"""
import sys
sys.path.insert(0, '/opt/trn_rl_repo')
import numpy as np
from contextlib import ExitStack

import concourse.bacc as bacc
import concourse.tile as tile
from concourse import mybir
from concourse.bass_utils import run_bass_kernel_spmd
from concourse import bass_isa
import ml_dtypes

F32 = mybir.dt.float32
F32R = mybir.dt.float32r
BF16 = mybir.dt.bfloat16
AF = mybir.ActivationFunctionType
ALU = mybir.AluOpType
AX = mybir.AxisListType

B, C, H, W = 8, 256, 96, 96
HW = H * W
K16 = 16
NEG = 0.001
NCHUNK = 72
FCH = [(k * 480, 480) for k in range(19)] + [(9120, 96)]


def _round_f32r(a):
    b = np.ascontiguousarray(a, dtype=np.float32).view(np.uint32)
    lsb = (b >> np.uint32(12)) & np.uint32(1)
    r = (b + np.uint32(0x7FF) + lsb) & np.uint32(0xFFFFF000)
    return r.view(np.float32)


def _build_program():
    nc = bacc.Bacc("TRN2", target_bir_lowering=False, debug=False)

    def din(name, shape, dt):
        return nc.dram_tensor(name, shape, dt, kind="ExternalInput").ap()

    X = din("x", [C, HW], F32R)
    Wst = din("wst", [C, 18], F32)
    WstR = din("wstr", [C, 18], F32R)
    Whb = din("whb", [2, 128, C], F32R)
    Whx = din("whx", [2, 128, C], F32R)
    Wm1 = din("wm1", [2, 128, C], F32R)
    Wm2 = din("wm2", [2, 128, C], F32R)
    Whm1 = din("whm1", [2, 128, C], F32R)
    Whm2 = din("whm2", [2, 128, C], F32R)
    Whe = din("whe", [2, 128, C], BF16)
    Wf2d = din("wf2d", [4, C], BF16)
    Wf2c = din("wf2c", [4, 1], BF16)
    Wcol = din("wcol", [14, 16], BF16)
    IDN = din("idn", [128, 128], F32)

    Y = nc.dram_tensor("y", [C, HW], F32, kind="ExternalOutput").ap()

    def lrelu(out, src):
        nc.scalar.activation(out, src, AF.Prelu, alpha=NEG)

    with tile.TileContext(nc) as tc, ExitStack() as ctx:
        per = ctx.enter_context(tc.tile_pool(name="per", bufs=1))
        x0 = per.tile([128, HW], F32R, tag="x0")
        x1 = per.tile([128, HW], F32R, tag="x1")
        nc.sync.dma_start(x0, X[0:128, :])
        nc.sync.dma_start(x1, X[128:256, :])
        xs = [x0, x1]

        idn = per.tile([128, 128], F32, tag="idn")
        nc.sync.dma_start(idn, IDN)
        wst, wstr = [], []
        for ct in range(2):
            t = per.tile([128, 18], F32, tag=f"wst{ct}", name=f"wst{ct}")
            nc.sync.dma_start(t, Wst[ct * 128:(ct + 1) * 128, :])
            wst.append(t)
            t2 = per.tile([128, 18], F32R, tag=f"wstr{ct}", name=f"wstr{ct}")
            nc.sync.dma_start(t2, WstR[ct * 128:(ct + 1) * 128, :])
            wstr.append(t2)

        def load_w(name, ap, dt):
            ts = []
            for kt in range(2):
                t = per.tile([128, C], dt, tag=f"{name}{kt}", name=f"{name}{kt}")
                nc.sync.dma_start(t, ap[kt])
                ts.append(t)
            return ts

        whb = load_w("whb", Whb, F32R)
        whx = load_w("whx", Whx, F32R)
        wm1 = load_w("wm1", Wm1, F32R)
        wm2 = load_w("wm2", Wm2, F32R)
        whm1 = load_w("whm1", Whm1, F32R)
        whm2 = load_w("whm2", Whm2, F32R)
        whe = load_w("whe", Whe, BF16)
        wf2d = per.tile([4, C], BF16, tag="wf2d")
        wf2c = per.tile([4, 1], BF16, tag="wf2c")
        wcol = per.tile([14, 16], BF16, tag="wcol")
        nc.sync.dma_start(wf2d, Wf2d)
        nc.sync.dma_start(wf2c, Wf2c)
        nc.sync.dma_start(wcol, Wcol)

        stats = per.tile([18, HW], F32R, tag="stats")
        S_sb = per.tile([K16, C], F32R, tag="S_sb")
        sspT_bf = per.tile([96, 96], BF16, tag="sspT")

        # ---------------- stage A: stats rows (f32r) ----------------
        sbLate = ctx.enter_context(tc.tile_pool(name="sbLate", bufs=1))
        pl6 = sbLate.tile([96, 6, 96], F32, tag="pl6")   # cm, bm_raw, bm, bmT, l, sex
        sv4 = sbLate.tile([96, 4], F32, tag="sv4")
        ssp = sbLate.tile([96, 96], F32, tag="ssp")
        f4r = sbLate.tile([4, HW], BF16, tag="f4r")

        with tc.tile_pool(name="psA", bufs=2, space="PSUM") as psA:
            for t in range(18):
                n0 = t * 512
                ps = psA.tile([18, 512], F32, tag="ps")
                nc.tensor.matmul(ps, wstr[0], x0[:, n0:n0 + 512], start=True, stop=False)
                nc.tensor.matmul(ps, wstr[1], x1[:, n0:n0 + 512], start=False, stop=True)
                nc.vector.tensor_copy(stats[:, n0:n0 + 512], ps)

        # ---------------- stage C: x transposes + fp32 S logits ----------------
        with tc.tile_pool(name="psC", bufs=2, space="PSUM") as psC, \
             tc.tile_pool(name="psS", bufs=2, space="PSUM") as psS, \
             tc.tile_pool(name="sbC", bufs=2) as sbC:
            sacc = [psS.tile([128, K16], F32, tag="sacc", name=f"sacc{j}") for j in range(2)]
            for i in range(NCHUNK):
                n0 = i * 128
                xt = sbC.tile([128, C], F32, tag="xt", bufs=3)
                for ct in range(2):
                    pt = psC.tile([128, 128], F32, tag="pt")
                    nc.tensor.transpose(pt, xs[ct][:, n0:n0 + 128].bitcast(F32), idn)
                    nc.vector.tensor_copy(xt[:, ct * 128:(ct + 1) * 128], pt)
                pst = psC.tile([128, K16], F32, tag="pst")
                nc.tensor.matmul(pst, xs[0][:, n0:n0 + 128].bitcast(F32), wst[0][:, 0:K16],
                                 start=True, stop=False)
                nc.tensor.matmul(pst, xs[1][:, n0:n0 + 128].bitcast(F32), wst[1][:, 0:K16],
                                 start=False, stop=True)
                st = sbC.tile([128, K16], F32, tag="st", bufs=3)
                nc.scalar.copy(st, pst)
                for mt in range(2):
                    nc.tensor.matmul(sacc[mt], xt[:, mt * 128:(mt + 1) * 128], st,
                                     start=(i == 0), stop=(i == NCHUNK - 1))

            s_pre = sbC.tile([K16, C], F32, tag="s_pre", bufs=1)
            for mt in range(2):
                sl = sbC.tile([128, K16], F32, tag="sl")
                nc.vector.tensor_copy(sl, sacc[mt])
                pt2 = psC.tile([K16, 128], F32, tag="pt2")
                nc.tensor.transpose(pt2, sl, idn)
                nc.scalar.copy(s_pre[:, mt * 128:(mt + 1) * 128], pt2)

            sm4 = sbC.tile([K16, 4], F32, tag="sm4", bufs=1)
            ex = sbC.tile([K16, C], F32, tag="ex", bufs=1)
            nc.vector.reduce_max(sm4[:, 0:1], s_pre, axis=AX.X)
            nc.vector.tensor_scalar_mul(sm4[:, 1:2], sm4[:, 0:1], -1.0)
            nc.scalar.activation(ex, s_pre, AF.Exp, bias=sm4[:, 1:2], scale=1.0)
            nc.vector.reduce_sum(sm4[:, 2:3], ex, axis=AX.X)
            nc.vector.reciprocal(sm4[:, 3:4], sm4[:, 2:3])
            nc.vector.tensor_scalar_mul(S_sb, ex, sm4[:, 3:4])

        # ---------------- spatial: max, Col, conv, planes, F4 ----------------
        with tc.tile_pool(name="sbS1", bufs=1) as sbS1:
            col = sbS1.tile([14, HW], BF16, tag="col")
            nc.gpsimd.memset(col, 0.0)
            max_tmp = sbS1.tile([64, HW], BF16, tag="max_tmp")
            nc.vector.tensor_tensor(max_tmp, x0[0:64, :].bitcast(F32),
                                    x0[64:128, :].bitcast(F32), op=ALU.max)
            nc.vector.tensor_tensor(max_tmp, max_tmp, x1[0:64, :].bitcast(F32), op=ALU.max)
            nc.vector.tensor_tensor(max_tmp, max_tmp, x1[64:128, :].bitcast(F32), op=ALU.max)
            # channel max: all-reduce across 64 partitions (in place), row 0 -> Col row 10
            nc.gpsimd.partition_all_reduce(max_tmp, max_tmp, channels=64,
                                           reduce_op=bass_isa.ReduceOp.max)
            nc.sync.dma_start(col[10:11, :], max_tmp[0:1, :])

            avg_ap = stats[16:17, :].bitcast(F32)
            for dy in range(-3, 4):
                r = dy + 3
                s0, s1 = max(0, -dy * 96), HW - max(0, dy * 96)
                nc.gpsimd.dma_start(out=col[r:r + 1, s0:s1], in_=avg_ap[:, s0 + dy * 96:s1 + dy * 96])
                if dy != 0:
                    nc.sync.dma_start(out=col[r + 7:r + 8, s0:s1], in_=col[10:11, s0 + dy * 96:s1 + dy * 96])

            c16 = sbS1.tile([16, HW], BF16, tag="c16")
            with tc.tile_pool(name="psCv", bufs=2, space="PSUM") as psCv:
                for t in range(18):
                    n0 = t * 512
                    ps = psCv.tile([16, 512], F32, tag="cps")
                    nc.tensor.matmul(ps, wcol, col[:, n0:n0 + 512], start=True, stop=True)
                    nc.vector.tensor_copy(c16[:, n0:n0 + 512], ps)

            c16_pl = sbS1.tile([96, 16, 96], BF16, tag="c16_pl")
            for r in range(16):
                nc.sync.dma_start(out=c16_pl[:, r, :],
                                  in_=c16[r:r + 1, :].rearrange("q (h w) -> q h w", w=96))
            accs = sbS1.tile([96, 4, 96], F32, tag="accs")
            nc.vector.memset(accs, 0.0)
            f4_pl = sbS1.tile([96, 4, 96], BF16, tag="f4_pl")
            colmap = [[0], [-1, 0, 1], [-2, -1, 0, 1, 2], [-3, -2, -1, 0, 1, 2, 3]]
            rr = 0
            for k, dxs in enumerate(colmap):
                acc = accs[:, k, :]
                first = True
                for dx in dxs:
                    a0, a1 = max(0, -dx), 96 - max(0, dx)
                    src = c16_pl[:, rr, a0 + dx:a1 + dx]
                    if first:
                        nc.vector.tensor_copy(acc[:, a0:a1], src)
                        first = False
                    else:
                        nc.vector.tensor_tensor(acc[:, a0:a1], acc[:, a0:a1], src, op=ALU.add)
                    rr += 1
                lrelu(f4_pl[:, k, :], acc)
            for k in range(4):
                nc.sync.dma_start(out=f4r[k:k + 1, :].rearrange("q (h w) -> q h w", w=96),
                                  in_=f4_pl[:, k, :])

        # ---------------- Esp production + fusion ----------------
        with tc.tile_pool(name="sbEsp", bufs=1) as sbEsp:
            espT = [sbEsp.tile([128, HW], BF16, tag=f"espT{ch}", name=f"espT{ch}")
                    for ch in range(2)]

            with tc.tile_pool(name="sbS2", bufs=1) as sbS2, \
                 tc.tile_pool(name="psL", bufs=2, space="PSUM") as psL:
                # Cm streamed straight into its plane (480-px chunks = 5 h-rows)
                for t in range(20):
                    n0, n = FCH[t]
                    h0, hn = n0 // 96, n // 96
                    ps = psL.tile([1, 480], F32, tag="cmps")
                    nc.tensor.matmul(ps[:, 0:n], wf2c, f4r[:, n0:n0 + n], start=True, stop=True)
                    cr = sbS2.tile([1, 480], F32, tag="cr", bufs=2)
                    lrelu(cr[:, 0:n], ps[:, 0:n])
                    nc.gpsimd.dma_start(out=pl6[h0:h0 + hn, 0, :],
                                        in_=cr[:, 0:n].rearrange("q (h w) -> q h w", w=96))

                nc.sync.dma_start(out=pl6[:, 1, :],
                                  in_=stats[17:18, :].bitcast(F32).rearrange("q (h w) -> q h w", w=96))
                lrelu(pl6[:, 2, :], pl6[:, 1, :])
                bmT_ps = psL.tile([96, 96], F32, tag="bmT_ps")
                nc.tensor.transpose(bmT_ps, pl6[:, 2, :], idn[0:96, 0:96])
                nc.vector.tensor_copy(pl6[:, 3, :], bmT_ps)

                l_ps = psL.tile([96, 96], F32, tag="l_ps")
                nc.tensor.matmul(l_ps, pl6[:, 3, :], pl6[:, 0, :], start=True, stop=True)
                nc.vector.tensor_copy(pl6[:, 4, :], l_ps)
                nc.vector.reduce_max(sv4[:, 0:1], pl6[:, 4, :], axis=AX.X)
                nc.vector.tensor_scalar_mul(sv4[:, 1:2], sv4[:, 0:1], -1.0)
                nc.scalar.activation(pl6[:, 5, :], pl6[:, 4, :], AF.Exp, bias=sv4[:, 1:2], scale=1.0)
                nc.vector.reduce_sum(sv4[:, 2:3], pl6[:, 5, :], axis=AX.X)
                nc.vector.reciprocal(sv4[:, 3:4], sv4[:, 2:3])
                nc.vector.tensor_scalar_mul(ssp, pl6[:, 5, :], sv4[:, 3:4])
                sspT_ps = psL.tile([96, 96], F32, tag="sspT_ps")
                nc.tensor.transpose(sspT_ps, ssp, idn[0:96, 0:96])
                nc.scalar.copy(sspT_bf, sspT_ps)

            with tc.tile_pool(name="sbDT", bufs=2) as sbDT, \
                 tc.tile_pool(name="psDT", bufs=2, space="PSUM") as psDT, \
                 tc.tile_pool(name="psE2", bufs=3, space="PSUM") as psE2:
                for w in range(96):
                    pd = psDT.tile([96, C], F32, tag="pd")
                    nc.tensor.matmul(pd, f4r[:, w::96], wf2d, start=True, stop=True)
                    dt_w = sbDT.tile([96, C], BF16, tag="dt_w")
                    lrelu(dt_w, pd)
                    for ch in range(2):
                        pe = psE2.tile([128, 96], F32, tag="pe")
                        nc.tensor.matmul(pe, dt_w[:, ch * 128:(ch + 1) * 128], sspT_bf,
                                         start=True, stop=True)
                        nc.vector.tensor_copy(espT[ch][:, w * 96:(w + 1) * 96], pe)

            # ------------- fusion (streamed 480-pixel chunks) -------------
            with tc.tile_pool(name="psF", bufs=1, space="PSUM") as psF, \
                 tc.tile_pool(name="sbFu", bufs=2) as sbFu:
                esp_v = [espT[ch].rearrange("p (w h) -> p h w", h=96) for ch in range(2)]
                for (n0, n) in FCH:
                    h0, hn = n0 // 96, n // 96
                    e_c = sbFu.tile([128, 2, 480], F32R, tag="e", bufs=1)
                    hh_c = sbFu.tile([128, 2, 480], F32R, tag="h")
                    m_c = sbFu.tile([128, 2, 480], F32R, tag="m")
                    for mt in range(2):
                        ps = psF.tile([128, 480], F32, tag=f"psE{mt}", name=f"psE{mt}")
                        nc.tensor.matmul(ps[:, 0:n], S_sb[:, mt * 128:(mt + 1) * 128],
                                         stats[0:K16, n0:n0 + n], start=True, stop=True)
                        nc.vector.tensor_copy(e_c[:, mt, 0:n], ps[:, 0:n])
                    for mt in range(2):
                        ms = slice(mt * 128, (mt + 1) * 128)
                        ps = psF.tile([128, 480], F32, tag=f"psH{mt}", name=f"psH{mt}")
                        nc.tensor.matmul(ps[:, 0:n], whb[0][:, ms], e_c[:, 0, 0:n], start=True, stop=False)
                        nc.tensor.matmul(ps[:, 0:n], whb[1][:, ms], e_c[:, 1, 0:n], start=False, stop=False)
                        for kt2 in range(2):
                            nc.tensor.matmul(ps[:, 0:n], whe[kt2][:, ms],
                                             esp_v[kt2][:, h0:h0 + hn, :], start=False, stop=False)
                        nc.tensor.matmul(ps[:, 0:n], whx[0][:, ms], x0[:, n0:n0 + n], start=False, stop=False)
                        nc.tensor.matmul(ps[:, 0:n], whx[1][:, ms], x1[:, n0:n0 + n], start=False, stop=True)
                        lrelu(hh_c[:, mt, 0:n], ps[:, 0:n])
                    for mt in range(2):
                        ms = slice(mt * 128, (mt + 1) * 128)
                        ps = psF.tile([128, 480], F32, tag=f"psM{mt}", name=f"psM{mt}")
                        nc.tensor.matmul(ps[:, 0:n], wm1[0][:, ms], hh_c[:, 0, 0:n], start=True, stop=False)
                        nc.tensor.matmul(ps[:, 0:n], wm1[1][:, ms], hh_c[:, 1, 0:n], start=False, stop=False)
                        nc.tensor.matmul(ps[:, 0:n], wm2[0][:, ms], x0[:, n0:n0 + n], start=False, stop=False)
                        nc.tensor.matmul(ps[:, 0:n], wm2[1][:, ms], x1[:, n0:n0 + n], start=False, stop=True)
                        nc.scalar.activation(m_c[:, mt, 0:n], ps[:, 0:n], AF.Sigmoid)
                    for mt in range(2):
                        ms = slice(mt * 128, (mt + 1) * 128)
                        ps = psF.tile([128, 480], F32, tag=f"psO{mt}", name=f"psO{mt}")
                        nc.tensor.matmul(ps[:, 0:n], whm1[0][:, ms], hh_c[:, 0, 0:n], start=True, stop=False)
                        nc.tensor.matmul(ps[:, 0:n], whm1[1][:, ms], hh_c[:, 1, 0:n], start=False, stop=False)
                        nc.tensor.matmul(ps[:, 0:n], whm2[0][:, ms], m_c[:, 0, 0:n], start=False, stop=False)
                        nc.tensor.matmul(ps[:, 0:n], whm2[1][:, ms], m_c[:, 1, 0:n], start=False, stop=True)
                        oc = sbFu.tile([128, 480], F32, tag="oc")
                        lrelu(oc[:, 0:n], ps[:, 0:n])
                        nc.sync.dma_start(Y[mt * 128:(mt + 1) * 128, n0:n0 + n], oc[:, 0:n])

    if not nc.is_finalized():
        nc.finalize()
    return nc


def _host_weights(w_f, w_beta, w1, w3, w5, w7, w_a2b, w_f2c, w_f2d, w_e, w_h, w_m, w_hm):
    bf = ml_dtypes.bfloat16
    wst = np.concatenate([w_f.T, w_a2b.T, np.full((C, 1), 1.0 / C, np.float32)], axis=1).astype(np.float32)

    def kt(mat):
        return _round_f32r(np.ascontiguousarray(mat.reshape(2, 128, -1)))

    w_h1, w_h2 = w_h[:, :C], w_h[:, C:]
    wcol = np.zeros((14, 16), np.float32)
    colbase = [0, 1, 4, 9]
    for ki, wk in enumerate([w1, w3, w5, w7]):
        p = (wk.shape[2] - 1) // 2
        for ci in range(2):
            for dy in range(-p, p + 1):
                for dx in range(-p, p + 1):
                    wcol[ci * 7 + dy + 3, colbase[ki] + dx + p] = wk[0, ci, dy + p, dx + p]
    return dict(
        wst=wst, wstr=_round_f32r(wst),
        whb=kt((w_h1 @ w_beta).T), whx=kt((w_h1 + w_h2).T),
        wm1=kt(w_m[:, :C].T), wm2=kt(w_m[:, C:].T),
        whm1=kt(w_hm[:, :C].T), whm2=kt(w_hm[:, C:].T),
        whe=np.ascontiguousarray((w_h2 @ w_e).T.reshape(2, 128, C)).astype(bf),
        wf2d=w_f2d.T.astype(bf), wf2c=w_f2c.T.astype(bf), wcol=wcol.astype(bf),
        idn=np.eye(128, dtype=np.float32),
    )


_NC_CACHE = {}


def kernel(x, w_f, w_beta, w1, w3, w5, w7, w_a2b, w_f2c, w_f2d, w_e, w_h, w_m, w_hm,
           _trace=False):
    if "nc" not in _NC_CACHE:
        _NC_CACHE["nc"] = _build_program()
    nc = _NC_CACHE["nc"]

    args = [np.asarray(a, np.float32) for a in
            (w_f, w_beta, w1, w3, w5, w7, w_a2b, w_f2c, w_f2d, w_e, w_h, w_m, w_hm)]
    wts = _host_weights(*args)
    xr = _round_f32r(np.asarray(x, np.float32).reshape(B, C, HW))
    in_maps = [dict(wts, x=np.ascontiguousarray(xr[i])) for i in range(B)]

    kw = dict(trace=True, trace_cores=[0]) if _trace else {}
    r = run_bass_kernel_spmd(nc, in_maps, list(range(B)), **kw)
    out = np.stack([r.results[i]["y"].reshape(C, H, W) for i in range(B)])
    if _trace:
        kernel._last = r
    return out
